# revision 5
# baseline (speedup 1.0000x reference)
"""Trainium2 Bass kernel for nn_CorrelationLayer.

Strategy: shard the 32 landmarks across 8 NeuronCores (4 each). The host
(this function) does the cheap data-dependent gather/scatter that defines
each landmark's 39x51 crop; each core runs the heavy correlation math for
its 4 landmarks:

    S^T = K^T Q          (1989x1989, contraction over 4 channels; PE,
                          2-way row-tiled f32r matmuls)
    E   = exp(S^T)       (ScalarE/ACT — the bottleneck engine)
    U   = V'^T E         (V' = [vx; vy; 1]; PE, 4-way col-tiled f32r,
                          PSUM-accumulated over 16 q-blocks)

and returns U (3 x p per landmark, split in 4 col-tile bands). The host
divides U[0:2]/U[2] (softmax normalization) and scatters the per-landmark
flow patches into the full-resolution outputs.
"""
import os
import numpy as np
from contextlib import ExitStack

import concourse.bass as bass
import concourse.bacc as bacc
import concourse.tile as tile
from concourse import mybir
from concourse.bass_utils import run_bass_kernel_spmd

# ---------------------------------------------------------------- constants
INIT_SCALE = 5
H, W = 192, 256
MASK_H, MASK_W = 39, 51
L = MASK_H * MASK_W          # 1989
LP = 2048                    # q/p padded to 16*128
NQB = LP // 128              # 16 q-blocks
NLM = 4                      # landmarks per core
NCORES = 8
F32 = mybir.dt.float32
F32R = mybir.dt.float32r
BF16 = mybir.dt.bfloat16


def _steps(n_full):
    half = [i + 1 for i in range(0, int(n_full) + 1, 2)]
    return list(reversed(half[1:])) + half


STEP_H = np.array(_steps(H / INIT_SCALE), dtype=np.float32)  # len 39
STEP_W = np.array(_steps(W / INIT_SCALE), dtype=np.float32)  # len 51


def _sigmoid(x):
    out = np.empty_like(x, dtype=np.float32)
    pos = x >= 0
    out[pos] = 1.0 / (1.0 + np.exp(-x[pos]))
    ex = np.exp(x[~pos])
    out[~pos] = ex / (1.0 + ex)
    return out


# ---------------------------------------------------------------- device
_PROGRAM = None


def _build_program():
    nc = bacc.Bacc(
        "TRN2", target_bir_lowering=False, debug=False, num_devices=NCORES
    )
    k_in = nc.dram_tensor("k_in", [NLM, 4, LP], F32R, kind="ExternalInput").ap()
    q_in = nc.dram_tensor("q_in", [NLM, 4, LP], F32R, kind="ExternalInput").ap()
    v_in = nc.dram_tensor("v_in", [NLM, 128, 3 * NQB], BF16, kind="ExternalInput").ap()
    u_out = nc.dram_tensor("u_out", [NLM, 2, 12, 1024], F32, kind="ExternalOutput").ap()

    EXP = mybir.ActivationFunctionType.Exp

    with tile.TileContext(nc) as tc, ExitStack() as ctx:
        kq = ctx.enter_context(tc.tile_pool(name="kq", bufs=2))
        vpool = ctx.enter_context(tc.tile_pool(name="vpool", bufs=2))
        epool = ctx.enter_context(tc.tile_pool(name="epool", bufs=3))
        stg = ctx.enter_context(tc.tile_pool(name="stg", bufs=2))
        stp = ctx.enter_context(tc.tile_pool(name="stp", bufs=3, space="PSUM"))
        fup = ctx.enter_context(tc.tile_pool(name="fup", bufs=1, space="PSUM"))

        for lm in range(NLM):
            krep = kq.tile([128, LP], F32R, tag="krep")
            qrep = kq.tile([128, LP], F32R, tag="qrep")
            v3t = vpool.tile([128, 3 * NQB], BF16)
            for b in range(2):
                nc.sync.dma_start(out=krep[32 * b:32 * b + 4, :], in_=k_in[lm])
                nc.sync.dma_start(out=qrep[32 * b:32 * b + 4, :], in_=q_in[lm])
            nc.sync.dma_start(out=v3t[:, :], in_=v_in[lm])

            for ph in range(2):
                fu = fup.tile([128, 1024], F32)
                nc.vector.memset(fu[:, :], 0.0)
                st_tiles = {}

                def mm1(qi):
                    st = stp.tile([128, 1024], F32, tag="st")
                    b = qi % 2
                    for n in range(2):
                        nc.tensor.matmul(
                            st[:, n * 512:(n + 1) * 512],
                            lhsT=krep[32 * b:32 * b + 4,
                                      qi * 128:(qi + 1) * 128],
                            rhs=qrep[32 * b:32 * b + 4,
                                     ph * 1024 + n * 512:
                                     ph * 1024 + (n + 1) * 512],
                            start=True, stop=True,
                            tile_position=(32 * b, 0),
                        )
                    st_tiles[qi] = st

                mm1(0)
                for qi in range(NQB):
                    if qi + 1 < NQB:
                        mm1(qi + 1)   # fill next S^T tile while ACT works
                    st = st_tiles.pop(qi)
                    e = epool.tile([128, 1024], BF16)
                    nc.scalar.activation(e[:, :], st[:, :], EXP)
                    j = qi % 4
                    for n in range(2):
                        nc.tensor.matmul(
                            fu[32 * j:32 * j + 3, n * 512:(n + 1) * 512],
                            lhsT=v3t[:, 3 * qi:3 * qi + 3],
                            rhs=e[:, n * 512:(n + 1) * 512],
                            start=(qi < 4), stop=(qi >= NQB - 4),
                            tile_position=(0, 32 * j),
                        )
                stag = stg.tile([128, 1024], F32)
                nc.vector.tensor_copy(out=stag[0:99, :], in_=fu[0:99, :])
                for j in range(4):
                    nc.sync.dma_start(
                        out=u_out[lm, ph, 3 * j:3 * j + 3, :],
                        in_=stag[32 * j:32 * j + 3, :],
                    )
    nc.compile()
    return nc


def _get_program():
    global _PROGRAM
    if _PROGRAM is None:
        _PROGRAM = _build_program()
    return _PROGRAM


# ---------------------------------------------------------------- host math
def _host_prep(location, fea_c, fea_p, scale_param, c_landmark, p_landmark,
               w1, b1, a1, w2, b2, a2):
    location = np.asarray(location, np.float32)
    fea_c = np.asarray(fea_c, np.float32)
    fea_p = np.asarray(fea_p, np.float32)
    scale_param = np.asarray(scale_param, np.float32)
    c_landmark = np.asarray(c_landmark)
    p_landmark = np.asarray(p_landmark)
    w1 = np.asarray(w1, np.float32); b1 = np.asarray(b1, np.float32)
    w2 = np.asarray(w2, np.float32); b2 = np.asarray(b2, np.float32)
    a1 = np.float32(a1); a2 = np.float32(a2)

    def reduce_small(x, w, b, a):
        xa = np.maximum(x[0], 0.0) + a * np.minimum(x[0], 0.0)
        return np.einsum('chw,oc->ohw', xa, w, dtype=np.float32) + b[:, None, None]

    fc = reduce_small(fea_c, w1, b1, a1)   # (4, 48, 64)
    fp = reduce_small(fea_p, w2, b2, a2)

    init_h = np.float32(W / INIT_SCALE)    # swapped H/W, as in the source model
    init_w = np.float32(H / INIT_SCALE)
    adj_cw, adj_ch = scale_param[0, :, 0, 0], scale_param[0, :, 0, 1]
    adj_pw, adj_ph = scale_param[0, :, 1, 0], scale_param[0, :, 1, 1]

    def mask_vecs(adj_h, adj_w):
        h = init_h * _sigmoid(adj_h)
        w = init_w * _sigmoid(adj_w)
        sh = _sigmoid((h[:, None] - STEP_H[None, :]) * 2)
        sw = _sigmoid((w[:, None] - STEP_W[None, :]) * 2)
        return sh.astype(np.float32), sw.astype(np.float32)

    sh_c, sw_c = mask_vecs(adj_ch, adj_cw)
    sh_p, sw_p = mask_vecs(adj_ph, adj_pw)

    c_cx = c_landmark[0, :, 0].astype(np.int64)
    c_cy = c_landmark[0, :, 1].astype(np.int64)
    p_cx = p_landmark[0, :, 0].astype(np.int64)
    p_cy = p_landmark[0, :, 1].astype(np.int64)
    use_lm = ~(((c_cx == 0) & (c_cy == 0)) | ((p_cx == 0) & (p_cy == 0)))

    r_ar = np.arange(MASK_H)
    c_ar = np.arange(MASK_W)

    def crop(cx, cy):
        top = cy - (MASK_H + 1) // 2
        bottom = cy + MASK_H // 2
        left = cx - (MASK_W + 1) // 2
        right = cx + MASK_W // 2
        dy1 = np.maximum(-top, 0); dy2 = np.maximum(bottom - H, 0)
        dx1 = np.maximum(-left, 0); dx2 = np.maximum(right - W, 0)
        new_top = top + dy1; new_left = left + dx1
        nh = np.maximum(bottom - dy2 - new_top, 0)
        nw = np.maximum(right - dx2 - new_left, 0)
        rows = np.clip(new_top[:, None] + r_ar[None, :], 0, H - 1)
        cols = np.clip(new_left[:, None] + c_ar[None, :], 0, W - 1)
        mrows = np.clip(dy1[:, None] + r_ar[None, :], 0, MASK_H - 1)
        mcols = np.clip(dx1[:, None] + c_ar[None, :], 0, MASK_W - 1)
        valid = (use_lm[:, None, None]
                 & (r_ar[None, :, None] < nh[:, None, None])
                 & (c_ar[None, None, :] < nw[:, None, None]))
        return rows, cols, mrows, mcols, valid

    rows_c, cols_c, mrows_c, mcols_c, valid_c = crop(c_cx, c_cy)
    rows_p, cols_p, mrows_p, mcols_p, valid_p = crop(p_cx, p_cy)

    def gather_patch(fsmall, sh, sw, rows, cols, mrows, mcols, valid):
        g = fsmall[:, (rows // 4)[:, :, None], (cols // 4)[:, None, :]]
        g = g.transpose(1, 0, 2, 3)                       # (32,4,39,51)
        m = (np.take_along_axis(sh, mrows, axis=1)[:, :, None]
             * np.take_along_axis(sw, mcols, axis=1)[:, None, :])
        return np.where(valid[:, None], g * m[:, None], np.float32(0.0)).astype(np.float32)

    cloth = gather_patch(fc, sh_c, sw_c, rows_c, cols_c, mrows_c, mcols_c, valid_c)
    person = gather_patch(fp, sh_p, sw_p, rows_p, cols_p, mrows_p, mcols_p, valid_p)

    loc5 = location.reshape(32, 2, H, W)
    lm_idx = np.arange(32)
    loc_g = loc5[lm_idx[:, None, None, None], np.arange(2)[None, :, None, None],
                 rows_c[:, None, :, None], cols_c[:, None, None, :]]
    loc_patch = np.where(valid_c[:, None], loc_g, np.float32(-1.0)).astype(np.float32)

    Q = person.reshape(32, 4, L)
    K = cloth.reshape(32, 4, L)
    V = loc_patch.reshape(32, 2, L)
    ones = np.ones((32, 1, L), np.float32)
    V3 = np.concatenate([V, ones], axis=1)               # (32, 3, L)
    return Q, K, V3, rows_p, cols_p, valid_p


def _host_finish(flows, rows_p, cols_p, valid_p):
    flow_patch = flows.reshape(32, 2, MASK_H, MASK_W)
    rr = np.where(valid_p, np.broadcast_to(rows_p[:, :, None], valid_p.shape), H)
    cc = np.where(valid_p, np.broadcast_to(cols_p[:, None, :], valid_p.shape), W)
    ii = np.arange(32)[:, None, None]

    flow_pad = np.full((32, H + 1, W + 1, 2), -1.0, np.float32)
    flow_pad[ii, rr, cc] = flow_patch.transpose(0, 2, 3, 1)
    landmark_flow = flow_pad[:, :H, :W, :].transpose(0, 3, 1, 2).reshape(1, 64, H, W)

    mask = np.zeros((32, H + 1, W + 1), np.float32)
    mask[ii, rr, cc] = 1.0
    mask = mask[None, :, :H, :W]
    return landmark_flow, mask


# ---------------------------------------------------------------- entry
_LAST_RESULTS = None   # for test harness introspection (exec_time_ns etc.)


def kernel(location, fea_c, fea_p, scale_param, H=H, W=W,
           c_landmark=None, p_landmark=None,
           w1=None, b1=None, a1=None, w2=None, b2=None, a2=None):
    global _LAST_RESULTS
    Q, K, V3, rows_p, cols_p, valid_p = _host_prep(
        location, fea_c, fea_p, scale_param, c_landmark, p_landmark,
        w1, b1, a1, w2, b2, a2)

    in_maps = []
    for c in range(NCORES):
        sl = slice(4 * c, 4 * c + 4)
        k_pad = np.zeros((NLM, 4, LP), np.float32)
        q_pad = np.zeros((NLM, 4, LP), np.float32)
        v_pad = np.zeros((NLM, LP, 3), np.float32)
        k_pad[:, :, :L] = K[sl]
        q_pad[:, :, :L] = Q[sl]
        v_pad[:, :L, :] = V3[sl].transpose(0, 2, 1)
        v_in = (v_pad.reshape(NLM, NQB, 128, 3)
                .transpose(0, 2, 1, 3).reshape(NLM, 128, 3 * NQB))
        import ml_dtypes
        v_in = np.ascontiguousarray(v_in).astype(ml_dtypes.bfloat16)
        in_maps.append({'k_in': k_pad, 'q_in': q_pad, 'v_in': v_in})

    nc = _get_program()
    trace = bool(int(os.environ.get("KERNEL_TRACE", "0")))
    res = run_bass_kernel_spmd(nc, in_maps, list(range(NCORES)), trace=trace)
    _LAST_RESULTS = res

    flows = np.empty((32, 2, L), np.float32)
    for c in range(NCORES):
        u = res.results[c]['u_out']              # (4, 2, 12, 1024)
        u = u.reshape(NLM, 2, 4, 3, 1024).sum(axis=2)     # (4, 2, 3, 1024)
        u = u.transpose(0, 2, 1, 3).reshape(NLM, 3, LP)[:, :, :L]
        flows[4 * c:4 * c + 4] = u[:, 0:2, :] / u[:, 2:3, :]

    return _host_finish(flows, rows_p, cols_p, valid_p)
